# revision 19
# baseline (speedup 1.0000x reference)
"""Trainium2 Bass kernel for nn_DistanceLoss (retrieval_knn).

Computes 5-way logits from per-tuple Euclidean distances between
frame-pair embeddings of queries and a support set.

Math restructuring vs the reference:
  - emb[n,(i,j)] = relu(A[n,i] + B[n,j] + b) with A = x@W1.T, B = x@W2.T
    (W = [W1 | W2]); frame-level matmuls are 7.5x fewer FLOPs than
    embedding each of the 120 tuples separately.
  - min_u dist^2 = -2 * max_u (q.s - s^2/2 - q^2/2); sqrt deferred until
    after all min reductions; norms folded into the Gram PSUM via one
    K=2 matmul (lhsT = [ones; -q^2/2], rhs = [-s^2/2; ones]).

Matmul operands are bf16 (measured logits error vs fp32 reference
~4e-4); the final sqrt/mean run in fp32.

Sharding: queries split across 8 cores (32 each); support set, W, b and
the class mask replicated.  No collectives; host concatenates logits.
"""

import sys
from contextlib import ExitStack

for _p in ("/opt/trn_rl_repo", "/root/.axon_site/_ro/trn_rl_repo"):
    if _p not in sys.path:
        sys.path.append(_p)

import ml_dtypes
import numpy as np

from concourse import bacc, mybir, tile
from concourse.bass import broadcast_tensor_aps
from concourse.bass_utils import run_bass_kernel_spmd

F32 = mybir.dt.float32
BF16 = mybir.dt.bfloat16
RELU = mybir.ActivationFunctionType.Relu
COPY = mybir.ActivationFunctionType.Copy
SQRT = mybir.ActivationFunctionType.Sqrt
MAX = mybir.AluOpType.max
AXX = mybir.AxisListType.X

N_CORES = 8
NQ_TOT = 256
NQC = NQ_TOT // N_CORES    # queries per core
NS = 25                    # support samples
SEQ = 16
D = 2048                   # input dim per frame
H = 1024                   # embedding dim
T = 120                    # C(16,2) frame pairs
WAY = 5
KC = D // 128              # 16 contraction chunks per W half
MC = H // 128              # 8 h-chunks
QG = 8                     # queries per group
NGROUP = NQC // QG
NEG_BIG = -3.0e38          # empty-class sentinel; -2*NEG_BIG overflows to inf

# tuple (i,j), i<j, lexicographic; OFF[i] = first tuple index with first=i
OFF = [0]
for _i in range(15):
    OFF.append(OFF[-1] + (15 - _i))


def build_program():
    nc = bacc.Bacc("TRN2", target_bir_lowering=False, debug=False,
                   num_devices=N_CORES)

    qf_d = nc.dram_tensor("qf", [D, NQC * SEQ], BF16,
                          kind="ExternalInput").ap()
    sf_d = nc.dram_tensor("sf", [D, NS * SEQ], BF16,
                          kind="ExternalInput").ap()
    # W tiles pre-chunked on host: [m, kgrp, ksub, half, 128(d), 128(h)]
    w_d = nc.dram_tensor("w", [MC, KC // 4, 4, 2, 128, 128], BF16,
                         kind="ExternalInput").ap()
    b_d = nc.dram_tensor("b", [128, MC], F32, kind="ExternalInput").ap()
    mask_d = nc.dram_tensor("mask", [128, WAY, NS], F32,
                            kind="ExternalInput").ap()
    out_d = nc.dram_tensor("out", [1, NQC * WAY], F32,
                           kind="ExternalOutput").ap()

    with tile.TileContext(nc) as tc, ExitStack() as top:
        cpool = top.enter_context(tc.tile_pool(name="const", bufs=1))
        abpool = top.enter_context(tc.tile_pool(name="ab", bufs=1))
        sepool = top.enter_context(tc.tile_pool(name="sepool", bufs=1))

        ones = cpool.tile([128, 128], BF16)
        nc.vector.memset(ones[:, :], 1.0)
        onesf = cpool.tile([128, 1], F32)
        nc.vector.memset(onesf[:, :], 1.0)
        bt = cpool.tile([128, MC], F32)
        nc.sync.dma_start(bt[:, :], b_d)
        mask = cpool.tile([128, WAY, NS], F32)
        nc.sync.dma_start(mask[:, :, :], mask_d)
        # norm-fold operands padded to K=128 so the extra matmul runs at
        # full LDW/stream rate: row0 = -s2/2 (ones for q2L), row1 = ones
        # (-q2/2 for q2L), rows 2..127 = 0.
        s2L = cpool.tile([128, NS, T], BF16)
        nc.vector.memset(s2L[:, :, :], 0.0)
        nc.vector.memset(s2L[0:2, :, :], 1.0)

        qA = abpool.tile([128, MC, NQC, SEQ], BF16)
        qB = abpool.tile([128, MC, NQC, SEQ], BF16)
        se = sepool.tile([128, MC, NS, T], BF16)

        # ---- Phase M: frame matmuls + in-phase support expansion ----
        with (
            tc.tile_pool(name="frames", bufs=1) as fpool,
            tc.tile_pool(name="wtiles", bufs=6) as wpool,
            tc.tile_pool(name="sab", bufs=2) as sabpool,
            tc.tile_pool(name="pm", bufs=2, space="PSUM") as pm,
        ):
            qf = fpool.tile([128, KC, NQC * SEQ], BF16)
            nc.sync.dma_start(qf[:, :, :],
                              qf_d.rearrange("(k p) n -> p k n", p=128))
            sf = fpool.tile([128, KC, NS * SEQ], BF16)
            nc.sync.dma_start(sf[:, :, :],
                              sf_d.rearrange("(k p) n -> p k n", p=128))

            for m in range(MC):
                pAq = pm.tile([128, NQC, SEQ], F32, tag="pAq")
                pBq = pm.tile([128, NQC, SEQ], F32, tag="pBq")
                pAs = pm.tile([128, NS, SEQ], F32, tag="pAs")
                pBs = pm.tile([128, NS, SEQ], F32, tag="pBs")
                for kg in range(KC // 4):
                    w12 = wpool.tile([128, 4, 2, 128], BF16, tag="w12")
                    nc.sync.dma_start(
                        w12[:, :, :, :],
                        w_d[m, kg].rearrange("ks two p c -> p ks two c"))
                    for ks in range(4):
                        k = kg * 4 + ks
                        st, sp = k == 0, k == KC - 1
                        nc.tensor.matmul(pAq[:, :, :], w12[:, ks, 0],
                                         qf[:, k, :], start=st, stop=sp)
                        nc.tensor.matmul(pBq[:, :, :], w12[:, ks, 1],
                                         qf[:, k, :], start=st, stop=sp)
                        nc.tensor.matmul(pAs[:, :, :], w12[:, ks, 0],
                                         sf[:, k, :], start=st, stop=sp)
                        nc.tensor.matmul(pBs[:, :, :], w12[:, ks, 1],
                                         sf[:, k, :], start=st, stop=sp)
                nc.scalar.copy(qA[:, m], pAq[:, :, :])
                nc.scalar.copy(qB[:, m], pBq[:, :, :])
                sAm = sabpool.tile([128, NS, SEQ], BF16, tag="sA")
                nc.scalar.copy(sAm[:, :, :], pAs[:, :, :])
                sBm = sabpool.tile([128, NS, SEQ], BF16, tag="sB")
                nc.scalar.copy(sBm[:, :, :], pBs[:, :, :])
                # expand this chunk's support tuples + relu, in-phase
                for i in range(15):
                    c = 15 - i
                    a_ap, b_ap = broadcast_tensor_aps(
                        sAm[:, :, i:i + 1], sBm[:, :, i + 1:SEQ])
                    nc.gpsimd.tensor_add(se[:, m, :, OFF[i]:OFF[i] + c],
                                             a_ap, b_ap)
                nc.scalar.activation(se[:, m], se[:, m], RELU,
                                     bias=bt[:, m:m + 1], scale=1.0)

        # ---- query groups ----
        with (
            tc.tile_pool(name="qe", bufs=1) as qepool,
            tc.tile_pool(name="qtmp", bufs=2) as qtmp,
            tc.tile_pool(name="qsq", bufs=4) as qsqpool,
            tc.tile_pool(name="small", bufs=2) as spool,
            tc.tile_pool(name="plog", bufs=1, space="PSUM") as plp,
        ):
            plog = plp.tile([1, NQC * WAY], F32)
            qes, q2Ls = [], []
            # expand every group + per-group -q2/2 up front; the S2
            # colsums on PE then overlap this DVE/ACT work
            with tc.tile_pool(name="pq2", bufs=2, space="PSUM") as pq2:
                for g in range(NGROUP):
                    q0 = g * QG
                    qe = qepool.tile([128, MC, QG, 128], BF16, tag=f"qe{g}",
                                     name=f"qe{g}")
                    nc.vector.memset(qe[:, :, :, T:128], 0.0)
                    for i in range(15):
                        c = 15 - i
                        a_ap, b_ap = broadcast_tensor_aps(
                            qA[:, :, q0:q0 + QG, i:i + 1],
                            qB[:, :, q0:q0 + QG, i + 1:SEQ])
                        nc.vector.tensor_add(qe[:, :, :, OFF[i]:OFF[i] + c],
                                             a_ap, b_ap)
                    for m in range(MC):
                        nc.scalar.activation(qe[:, m], qe[:, m], RELU,
                                             bias=bt[:, m:m + 1], scale=1.0)

                    q2n = qtmp.tile([1, QG, 128], BF16, tag="q2n")
                    for blk in range(2):
                        c0 = blk * 4
                        p2 = pq2.tile([128, 4, 128], F32, tag="pq2")
                        for m in range(MC):
                            sq = qsqpool.tile([128, 4, 128], BF16, tag="qsq")
                            nc.vector.tensor_mul(sq[:, :, :],
                                                 qe[:, m, c0:c0 + 4],
                                                 qe[:, m, c0:c0 + 4])
                            nc.tensor.matmul(p2[:, :, :], ones[:, :],
                                             sq[:, :, :],
                                             start=(m == 0),
                                             stop=(m == MC - 1))
                        nc.scalar.activation(q2n[0:1, c0:c0 + 4],
                                             p2[0:1, :, :], COPY, scale=-0.5)
                    q2L = qtmp.tile([128, QG, 128], BF16, tag=f"q2L{g}",
                                    name=f"q2L{g}")
                    nc.vector.memset(q2L[:, :, :], 0.0)
                    nc.vector.memset(q2L[0:2, :, :], 1.0)
                    nc.sync.dma_start(q2L[1:2, :, :], q2n[0:1, :, :])
                    qes.append(qe)
                    q2Ls.append(q2L)

            # ---- S2: -s2/2 into s2L row 0 ----
            with (
                tc.tile_pool(name="sq", bufs=4) as sqpool,
                tc.tile_pool(name="ps2", bufs=1, space="PSUM") as ps2,
            ):
                s2ps = []
                for blk in range(7):
                    s2ps.append(ps2.tile([128, 4, T], F32, tag=f"ps2{blk}",
                                         name=f"s2ps{blk}"))
                for m in range(MC):
                    for blk in range(7):
                        s0 = blk * 4
                        ns = min(4, NS - s0)
                        sq = sqpool.tile([128, 4, T], BF16, tag="sq")
                        nc.vector.tensor_mul(sq[:, :ns, :],
                                             se[:, m, s0:s0 + ns],
                                             se[:, m, s0:s0 + ns])
                        nc.tensor.matmul(s2ps[blk][:, :ns, :], ones[:, :],
                                         sq[:, :ns, :],
                                         start=(m == 0), stop=(m == MC - 1))
                for blk in range(7):
                    s0 = blk * 4
                    ns = min(4, NS - s0)
                    nc.scalar.activation(s2L[0:1, s0:s0 + ns],
                                         s2ps[blk][0:1, :ns, :], COPY,
                                         scale=-0.5)

            with tc.tile_pool(name="pd", bufs=4, space="PSUM") as pdp:
              for g in range(NGROUP):
                q0 = g * QG
                qe = qes[g]
                q2L = q2Ls[g]
                for q in range(QG):
                    maxm = spool.tile([128, 1, NS], F32, tag="maxm")
                    for blk in range(7):
                        s0 = blk * 4
                        ns = min(4, NS - s0)
                        pdt = pdp.tile([128, 4, T], F32, tag="pd")
                        for m in range(MC):
                            nc.tensor.matmul(pdt[:, :ns, :],
                                             qe[:, m, q],
                                             se[:, m, s0:s0 + ns],
                                             start=(m == 0), stop=False)
                        nc.tensor.matmul(pdt[:, :ns, :], q2L[:, q],
                                         s2L[:, s0:s0 + ns],
                                         start=False, stop=True)
                        nc.vector.tensor_reduce(maxm[:, 0, s0:s0 + ns],
                                                pdt[:, :ns, :],
                                                axis=AXX, op=MAX)
                    masked = spool.tile([128, WAY, NS], F32, tag="masked")
                    mm_ap, mk_ap = broadcast_tensor_aps(maxm[:, 0:1, :],
                                                        mask[:, :, :])
                    nc.vector.tensor_add(masked[:, :, :], mm_ap, mk_ap)
                    mc_t = spool.tile([128, WAY], F32, tag="mc")
                    nc.vector.tensor_reduce(mc_t[:, :], masked[:, :, :],
                                            axis=AXX, op=MAX)
                    dt_ = spool.tile([128, WAY], F32, tag="d")
                    nc.vector.tensor_scalar(dt_[:, :], mc_t[:, :],
                                            -2.0, 1e-12,
                                            mybir.AluOpType.mult, MAX)
                    nc.scalar.activation(dt_[:, :], dt_[:, :], SQRT)
                    qi = q0 + q
                    nc.tensor.matmul(plog[0:1, qi * WAY:(qi + 1) * WAY],
                                     onesf[0:T, :], dt_[0:T, :],
                                     start=True, stop=True)

            louts = cpool.tile([1, NQC * WAY], F32)
            nc.scalar.activation(louts[:, :], plog[:, :], COPY,
                                 scale=-1.0 / T)
            nc.sync.dma_start(out_d, louts[:, :])
    nc.compile()
    return nc


_NC_CACHE = None
LAST = None


def kernel(support_set, queries, support_labels, W, b):
    global _NC_CACHE, LAST
    support_set = np.asarray(support_set, dtype=np.float32)
    queries = np.asarray(queries, dtype=np.float32)
    support_labels = np.asarray(support_labels)
    W = np.asarray(W, dtype=np.float32)
    b = np.asarray(b, dtype=np.float32)
    bf = ml_dtypes.bfloat16

    # host-side layout prep (pure data movement + bf16 cast)
    sf = np.ascontiguousarray(support_set.reshape(NS * SEQ, D).T.astype(bf))
    wt = np.ascontiguousarray(
        W.reshape(MC, 128, 2, KC // 4, 4, 128)
        .transpose(0, 3, 4, 2, 5, 1).astype(bf))
    bt = np.ascontiguousarray(b.reshape(MC, 128).T)
    maskv = np.where(support_labels[None, :] == np.arange(WAY)[:, None],
                     np.float32(0.0), np.float32(NEG_BIG)).astype(np.float32)
    maskrep = np.ascontiguousarray(
        np.broadcast_to(maskv[None], (128, WAY, NS)))

    in_maps = []
    for c in range(N_CORES):
        qfc = np.ascontiguousarray(
            queries[c * NQC:(c + 1) * NQC].reshape(NQC * SEQ, D).T.astype(bf))
        in_maps.append({"qf": qfc, "sf": sf, "w": wt, "b": bt,
                        "mask": maskrep})

    if _NC_CACHE is None:
        _NC_CACHE = build_program()
    res = run_bass_kernel_spmd(_NC_CACHE, in_maps, list(range(N_CORES)))
    LAST = res
    outs = [res.results[c]["out"].reshape(NQC, WAY) for c in range(N_CORES)]
    return np.concatenate(outs, axis=0)


if __name__ == "__main__":
    rng = np.random.default_rng(0)
    out = kernel(
        rng.standard_normal((NS, SEQ, D)).astype(np.float32),
        rng.standard_normal((NQ_TOT, SEQ, D)).astype(np.float32),
        (np.arange(NS) % WAY).astype(np.int32),
        (rng.standard_normal((H, 2 * D)) / np.sqrt(2 * D)).astype(np.float32),
        (rng.standard_normal(H) * 0.01).astype(np.float32),
    )
    print(out.shape, out[:2])


# revision 20
# speedup vs baseline: 1.2704x; 1.2704x over previous
"""Trainium2 Bass kernel for nn_DistanceLoss (retrieval_knn).

Computes 5-way logits from per-tuple Euclidean distances between
frame-pair embeddings of queries and a support set.

Math restructuring vs the reference:
  - emb[n,(i,j)] = relu(A[n,i] + B[n,j] + b) with A = x@W1.T, B = x@W2.T
    (W = [W1 | W2]); frame-level matmuls are 7.5x fewer FLOPs than
    embedding each of the 120 tuples separately.
  - min_u dist^2 = -2 * max_u (q.s - s^2/2 - q^2/2); sqrt deferred until
    after all min reductions; norms folded into the Gram PSUM via one
    K=128-padded bf16 matmul (row0/row1 carry [ones; -q^2/2] and
    [-s^2/2; ones]).

Embeddings are stored fp8e4m3 and the Gram runs as DoubleRow matmuls
(K=256 per instruction, 2 fp8 weights per PE cell); frame matmuls are
bf16; norms, sqrt and mean run in fp32-class precision.

Sharding: queries split across 8 cores (32 each); support set, W, b and
the class mask replicated.  No collectives; host concatenates logits.
"""

import sys
from contextlib import ExitStack

for _p in ("/opt/trn_rl_repo", "/root/.axon_site/_ro/trn_rl_repo"):
    if _p not in sys.path:
        sys.path.append(_p)

import ml_dtypes
import numpy as np

from concourse import bacc, mybir, tile
from concourse.bass import broadcast_tensor_aps
from concourse.bass_utils import run_bass_kernel_spmd

F32 = mybir.dt.float32
BF16 = mybir.dt.bfloat16
FP8 = mybir.dt.float8e4
DR = mybir.MatmulPerfMode.DoubleRow
RELU = mybir.ActivationFunctionType.Relu
COPY = mybir.ActivationFunctionType.Copy
SQRT = mybir.ActivationFunctionType.Sqrt
MAX = mybir.AluOpType.max
AXX = mybir.AxisListType.X

N_CORES = 8
NQ_TOT = 256
NQC = NQ_TOT // N_CORES    # queries per core
NS = 25                    # support samples
SEQ = 16
D = 2048                   # input dim per frame
H = 1024                   # embedding dim
T = 120                    # C(16,2) frame pairs
WAY = 5
KC = D // 128              # 16 contraction chunks per W half
MC = H // 128              # 8 h-chunks
QG = 8                     # queries per group
NGROUP = NQC // QG
NEG_BIG = -3.0e38          # empty-class sentinel; -2*NEG_BIG overflows to inf

# tuple (i,j), i<j, lexicographic; OFF[i] = first tuple index with first=i
OFF = [0]
for _i in range(15):
    OFF.append(OFF[-1] + (15 - _i))


def build_program():
    nc = bacc.Bacc("TRN2", target_bir_lowering=False, debug=False,
                   num_devices=N_CORES)

    qf_d = nc.dram_tensor("qf", [D, NQC * SEQ], BF16,
                          kind="ExternalInput").ap()
    sf_d = nc.dram_tensor("sf", [D, NS * SEQ], BF16,
                          kind="ExternalInput").ap()
    # W tiles pre-chunked on host: [m, kgrp, ksub, half, 128(d), 128(h)]
    w_d = nc.dram_tensor("w", [MC, KC // 4, 4, 2, 128, 128], BF16,
                         kind="ExternalInput").ap()
    b_d = nc.dram_tensor("b", [128, MC], F32, kind="ExternalInput").ap()
    mask_d = nc.dram_tensor("mask", [128, WAY, NS], F32,
                            kind="ExternalInput").ap()
    out_d = nc.dram_tensor("out", [1, NQC * WAY], F32,
                           kind="ExternalOutput").ap()

    with tile.TileContext(nc) as tc, ExitStack() as top:
        cpool = top.enter_context(tc.tile_pool(name="const", bufs=1))
        abpool = top.enter_context(tc.tile_pool(name="ab", bufs=1))
        sepool = top.enter_context(tc.tile_pool(name="sepool", bufs=1))

        ones = cpool.tile([128, 128], BF16)
        nc.vector.memset(ones[:, :], 1.0)
        onesf = cpool.tile([128, 1], F32)
        nc.vector.memset(onesf[:, :], 1.0)
        bt = cpool.tile([128, MC], F32)
        nc.sync.dma_start(bt[:, :], b_d)
        mask = cpool.tile([128, WAY, NS], F32)
        nc.sync.dma_start(mask[:, :, :], mask_d)
        # norm-fold operands padded to K=128 so the extra matmul runs at
        # full LDW/stream rate: row0 = -s2/2 (ones for q2L), row1 = ones
        # (-q2/2 for q2L), rows 2..127 = 0.
        s2L = cpool.tile([128, NS, T], BF16)
        nc.vector.memset(s2L[:, :, :], 0.0)
        nc.vector.memset(s2L[0:2, :, :], 1.0)

        qA = abpool.tile([128, MC, NQC, SEQ], BF16)
        qB = abpool.tile([128, MC, NQC, SEQ], BF16)
        se = sepool.tile([128, MC, NS, T], FP8)

        # ---- Phase M: frame matmuls + in-phase support expansion ----
        with (
            tc.tile_pool(name="frames", bufs=1) as fpool,
            tc.tile_pool(name="wtiles", bufs=6) as wpool,
            tc.tile_pool(name="sab", bufs=2) as sabpool,
            tc.tile_pool(name="pm", bufs=2, space="PSUM") as pm,
        ):
            qf = fpool.tile([128, KC, NQC * SEQ], BF16)
            nc.sync.dma_start(qf[:, :, :],
                              qf_d.rearrange("(k p) n -> p k n", p=128))
            sf = fpool.tile([128, KC, NS * SEQ], BF16)
            nc.sync.dma_start(sf[:, :, :],
                              sf_d.rearrange("(k p) n -> p k n", p=128))

            for m in range(MC):
                pAq = pm.tile([128, NQC, SEQ], F32, tag="pAq")
                pBq = pm.tile([128, NQC, SEQ], F32, tag="pBq")
                pAs = pm.tile([128, NS, SEQ], F32, tag="pAs")
                pBs = pm.tile([128, NS, SEQ], F32, tag="pBs")
                for kg in range(KC // 4):
                    w12 = wpool.tile([128, 4, 2, 128], BF16, tag="w12")
                    nc.sync.dma_start(
                        w12[:, :, :, :],
                        w_d[m, kg].rearrange("ks two p c -> p ks two c"))
                    for ks in range(4):
                        k = kg * 4 + ks
                        st, sp = k == 0, k == KC - 1
                        nc.tensor.matmul(pAq[:, :, :], w12[:, ks, 0],
                                         qf[:, k, :], start=st, stop=sp)
                        nc.tensor.matmul(pBq[:, :, :], w12[:, ks, 1],
                                         qf[:, k, :], start=st, stop=sp)
                        nc.tensor.matmul(pAs[:, :, :], w12[:, ks, 0],
                                         sf[:, k, :], start=st, stop=sp)
                        nc.tensor.matmul(pBs[:, :, :], w12[:, ks, 1],
                                         sf[:, k, :], start=st, stop=sp)
                nc.scalar.copy(qA[:, m], pAq[:, :, :])
                nc.scalar.copy(qB[:, m], pBq[:, :, :])
                sAm = sabpool.tile([128, NS, SEQ], BF16, tag="sA")
                nc.scalar.copy(sAm[:, :, :], pAs[:, :, :])
                sBm = sabpool.tile([128, NS, SEQ], BF16, tag="sB")
                nc.scalar.copy(sBm[:, :, :], pBs[:, :, :])
                # expand this chunk's support tuples + relu, in-phase
                for i in range(15):
                    c = 15 - i
                    a_ap, b_ap = broadcast_tensor_aps(
                        sAm[:, :, i:i + 1], sBm[:, :, i + 1:SEQ])
                    nc.gpsimd.tensor_add(se[:, m, :, OFF[i]:OFF[i] + c],
                                         a_ap, b_ap)
                nc.scalar.activation(se[:, m], se[:, m], RELU,
                                     bias=bt[:, m:m + 1], scale=1.0)

        # ---- Phase S2: -s2/2 into s2L row 0 ----
        with (
            tc.tile_pool(name="sq", bufs=4) as sqpool,
            tc.tile_pool(name="ps2", bufs=1, space="PSUM") as ps2,
        ):
            s2ps = []
            for blk in range(7):
                s2ps.append(ps2.tile([1, 4, T], F32, tag=f"ps2{blk}",
                                     name=f"s2ps{blk}"))
            for m in range(MC):
                for blk in range(7):
                    s0 = blk * 4
                    ns = min(4, NS - s0)
                    sq = sqpool.tile([128, 4, T], BF16, tag="sq")
                    nc.vector.tensor_mul(sq[:, :ns, :], se[:, m, s0:s0 + ns],
                                         se[:, m, s0:s0 + ns])
                    nc.tensor.matmul(s2ps[blk][:, :ns, :], ones[:, 0:1],
                                     sq[:, :ns, :],
                                     start=(m == 0), stop=(m == MC - 1))
            for blk in range(7):
                s0 = blk * 4
                ns = min(4, NS - s0)
                nc.scalar.activation(s2L[0:1, s0:s0 + ns],
                                     s2ps[blk][:, :ns, :], COPY, scale=-0.5)

        # ---- query groups ----
        with (
            tc.tile_pool(name="qe", bufs=2) as qepool,
            tc.tile_pool(name="qtmp", bufs=2) as qtmp,
            tc.tile_pool(name="qsq", bufs=4) as qsqpool,
            tc.tile_pool(name="small", bufs=2) as spool,
            tc.tile_pool(name="pq2", bufs=2, space="PSUM") as pq2,
            tc.tile_pool(name="pd", bufs=4, space="PSUM") as pdp,
            tc.tile_pool(name="plog", bufs=1, space="PSUM") as plp,
        ):
            plog = plp.tile([1, NQC * WAY], F32)
            for g in range(NGROUP):
                q0 = g * QG
                qe = qepool.tile([128, MC, QG, 128], FP8, tag="qe")
                nc.vector.memset(qe[:, :, :, T:128], 0.0)
                for i in range(15):
                    c = 15 - i
                    a_ap, b_ap = broadcast_tensor_aps(
                        qA[:, :, q0:q0 + QG, i:i + 1],
                        qB[:, :, q0:q0 + QG, i + 1:SEQ])
                    nc.vector.tensor_add(qe[:, :, :, OFF[i]:OFF[i] + c],
                                         a_ap, b_ap)
                for m in range(MC):
                    nc.scalar.activation(qe[:, m], qe[:, m], RELU,
                                         bias=bt[:, m:m + 1], scale=1.0)

                # -q2/2 for this group -> q2L row 1 (via DMA partition hop)
                q2n = qtmp.tile([1, QG, 128], BF16, tag="q2n")
                for blk in range(2):
                    c0 = blk * 4
                    p2 = pq2.tile([1, 4, 128], F32, tag="pq2")
                    for m in range(MC):
                        sq = qsqpool.tile([128, 4, 128], BF16, tag="qsq")
                        nc.vector.tensor_mul(sq[:, :, :],
                                             qe[:, m, c0:c0 + 4],
                                             qe[:, m, c0:c0 + 4])
                        nc.tensor.matmul(p2[:, :, :], ones[:, 0:1],
                                         sq[:, :, :],
                                         start=(m == 0), stop=(m == MC - 1))
                    nc.scalar.activation(q2n[0:1, c0:c0 + 4], p2[:, :, :],
                                         COPY, scale=-0.5)
                q2L = qtmp.tile([128, QG, 128], BF16, tag="q2L")
                nc.vector.memset(q2L[:, :, :], 0.0)
                nc.vector.memset(q2L[0:2, :, :], 1.0)
                nc.sync.dma_start(q2L[1:2, :, :], q2n[0:1, :, :])

                for q in range(QG):
                    maxm = spool.tile([128, 1, NS], F32, tag="maxm")
                    for blk in range(7):
                        s0 = blk * 4
                        ns = min(4, NS - s0)
                        pdt = pdp.tile([128, 4, T], F32, tag="pd")
                        for j in range(MC // 2):
                            nc.tensor.matmul(pdt[:, :ns, :],
                                             qe[:, 2 * j:2 * j + 2, q],
                                             se[:, 2 * j:2 * j + 2,
                                                s0:s0 + ns],
                                             start=(j == 0), stop=False,
                                             perf_mode=DR)
                        nc.tensor.matmul(pdt[:, :ns, :], q2L[:, q],
                                         s2L[:, s0:s0 + ns],
                                         start=False, stop=True)
                        nc.vector.tensor_reduce(maxm[:, 0, s0:s0 + ns],
                                                pdt[:, :ns, :],
                                                axis=AXX, op=MAX)
                    masked = spool.tile([128, WAY, NS], F32, tag="masked")
                    mm_ap, mk_ap = broadcast_tensor_aps(maxm[:, 0:1, :],
                                                        mask[:, :, :])
                    nc.vector.tensor_add(masked[:, :, :], mm_ap, mk_ap)
                    mc_t = spool.tile([128, WAY], F32, tag="mc")
                    nc.vector.tensor_reduce(mc_t[:, :], masked[:, :, :],
                                            axis=AXX, op=MAX)
                    dt_ = spool.tile([128, WAY], F32, tag="d")
                    nc.vector.tensor_scalar(dt_[:, :], mc_t[:, :],
                                            -2.0, 1e-12,
                                            mybir.AluOpType.mult, MAX)
                    nc.scalar.activation(dt_[:, :], dt_[:, :], SQRT)
                    qi = q0 + q
                    nc.tensor.matmul(plog[0:1, qi * WAY:(qi + 1) * WAY],
                                     onesf[0:T, :], dt_[0:T, :],
                                     start=True, stop=True)

            louts = cpool.tile([1, NQC * WAY], F32)
            nc.scalar.activation(louts[:, :], plog[:, :], COPY,
                                 scale=-1.0 / T)
            nc.sync.dma_start(out_d, louts[:, :])
    nc.compile()
    return nc


_NC_CACHE = None
LAST = None


def kernel(support_set, queries, support_labels, W, b):
    global _NC_CACHE, LAST
    support_set = np.asarray(support_set, dtype=np.float32)
    queries = np.asarray(queries, dtype=np.float32)
    support_labels = np.asarray(support_labels)
    W = np.asarray(W, dtype=np.float32)
    b = np.asarray(b, dtype=np.float32)
    bf = ml_dtypes.bfloat16

    # host-side layout prep (pure data movement + bf16 cast)
    sf = np.ascontiguousarray(support_set.reshape(NS * SEQ, D).T.astype(bf))
    wt = np.ascontiguousarray(
        W.reshape(MC, 128, 2, KC // 4, 4, 128)
        .transpose(0, 3, 4, 2, 5, 1).astype(bf))
    bt = np.ascontiguousarray(b.reshape(MC, 128).T)
    maskv = np.where(support_labels[None, :] == np.arange(WAY)[:, None],
                     np.float32(0.0), np.float32(NEG_BIG)).astype(np.float32)
    maskrep = np.ascontiguousarray(
        np.broadcast_to(maskv[None], (128, WAY, NS)))

    in_maps = []
    for c in range(N_CORES):
        qfc = np.ascontiguousarray(
            queries[c * NQC:(c + 1) * NQC].reshape(NQC * SEQ, D).T.astype(bf))
        in_maps.append({"qf": qfc, "sf": sf, "w": wt, "b": bt,
                        "mask": maskrep})

    if _NC_CACHE is None:
        _NC_CACHE = build_program()
    res = run_bass_kernel_spmd(_NC_CACHE, in_maps, list(range(N_CORES)))
    LAST = res
    outs = [res.results[c]["out"].reshape(NQC, WAY) for c in range(N_CORES)]
    return np.concatenate(outs, axis=0)


if __name__ == "__main__":
    rng = np.random.default_rng(0)
    out = kernel(
        rng.standard_normal((NS, SEQ, D)).astype(np.float32),
        rng.standard_normal((NQ_TOT, SEQ, D)).astype(np.float32),
        (np.arange(NS) % WAY).astype(np.int32),
        (rng.standard_normal((H, 2 * D)) / np.sqrt(2 * D)).astype(np.float32),
        (rng.standard_normal(H) * 0.01).astype(np.float32),
    )
    print(out.shape, out[:2])
